# revision 8
# baseline (speedup 1.0000x reference)
"""MultiHeadAttention (B=2, S=2048, D=1024, H=16, depth=64) on 8 trn2 cores.

Sharding: core c -> batch b=c//4, head-group g=c%4 (heads 4g..4g+3).
Per-core device program (SPMD, identical program, different inputs):
  - inputs pre-transposed on host: xq/xk/xv = x_b.T [1024, 2048]
  - Q/K/V projections computed feature-major: qhT/khT [256, 2048]
  - V transposed on-device (PE transpose) into seq-major interleaved tiles
    vI[sc] [128, 4, 65] with an all-ones column 64 per head, so attn@V also
    produces the softmax denominator (row 64 of ctx psum).
  - scoresT[k,q] = khT.T-slice @ qhT-slice, exp on ACT (scale 1/8, no max
    subtraction: scores ~ N(0,1), safe in fp32), attn@V accumulates
    ctx [65, 512] over 16 k-chunks.
  - normalization: reciprocal of row 64 (DVE) -> broadcast via rank-1 matmul
    -> multiply (DVE) into feature-major ctxN [256, 2048].
  - output projection -> outT partial [1024, 2048]; host sums the 4 head-group
    partials per batch, transposes back, adds bo.
All matmuls use float32r (1 cycle/row at moving-free >= 256). The BIR
verifier requires every f32r matmul operand to be PRODUCED by a compute op
with float32r output dtype (the engine rounds); DMA'd tensors are staged in
fp32 and round-copied (weights on DVE, activations on Pool).
"""

import numpy as np

B, S, D = 2, 2048, 1024
FG = 256  # features per core (4 heads x 64)

_compiled = None


def _build_program(repeat=1):
    import concourse.bass as bass  # noqa: F401
    import concourse.tile as tile
    from concourse import bacc, mybir, masks

    f32 = mybir.dt.float32
    f32r = mybir.dt.float32r
    EXP = mybir.ActivationFunctionType.Exp
    MULT = mybir.AluOpType.mult

    nc = bacc.Bacc("TRN2", target_bir_lowering=False, debug=False)

    xq_d = nc.dram_tensor("xq", [D, S], f32, kind="ExternalInput")
    xk_d = nc.dram_tensor("xk", [D, S], f32, kind="ExternalInput")
    xv_d = nc.dram_tensor("xv", [D, S], f32, kind="ExternalInput")
    wq_d = nc.dram_tensor("wq", [D, FG], f32, kind="ExternalInput")
    wk_d = nc.dram_tensor("wk", [D, FG], f32, kind="ExternalInput")
    wv_d = nc.dram_tensor("wv", [D, FG], f32, kind="ExternalInput")
    wo_d = nc.dram_tensor("wo", [FG, D], f32, kind="ExternalInput")
    bq_d = nc.dram_tensor("bq", [1, FG], f32, kind="ExternalInput")
    bk_d = nc.dram_tensor("bk", [1, FG], f32, kind="ExternalInput")
    bv_d = nc.dram_tensor("bv", [1, FG], f32, kind="ExternalInput")
    out_d = nc.dram_tensor("out", [D, S], f32, kind="ExternalOutput")

    with tile.TileContext(nc) as tc:
      for _rep in range(repeat):
        with tc.tile_pool(name="const", bufs=1) as cpool:
            onesf = cpool.tile([1, 512], f32, tag="onesf", name="onesf")
            nc.gpsimd.memset(onesf[:], 1.0)
            ones = cpool.tile([1, 512], f32r, tag="ones", name="ones")
            nc.vector.tensor_copy(ones[:], onesf[:])
            o41f = cpool.tile([128, 4, 1], f32, tag="o41f", name="o41f")
            nc.gpsimd.memset(o41f[:], 1.0)
            ones41 = cpool.tile([128, 4, 1], f32r, tag="ones41", name="ones41")
            nc.vector.tensor_copy(ones41[:], o41f[:])
            zbias = cpool.tile([128, 1], f32, tag="zbias", name="zbias")
            nc.gpsimd.memset(zbias[:], 0.0)
            ident = cpool.tile([128, 128], f32, tag="ident", name="ident")
            masks.make_identity(nc, ident[:])

            w_sb = {}
            wo_sb = []
            b_sb = {}
            with tc.tile_pool(name="wst", bufs=1) as wpool:
                for nm, d in (("wq", wq_d), ("wk", wk_d), ("wv", wv_d)):
                    for kk in range(8):
                        st = wpool.tile([128, FG], f32, name="ws", bufs=4)
                        nc.sync.dma_start(st[:], d.ap()[128 * kk:128 * (kk + 1), :])
                        t = cpool.tile([128, FG], f32r, tag=f"{nm}{kk}",
                                       name=f"{nm}{kk}")
                        nc.vector.tensor_copy(t[:], st[:])
                        w_sb[(nm, kk)] = t
                for kk2 in range(2):
                    st = wpool.tile([128, D], f32, name="wos", bufs=2)
                    nc.sync.dma_start(st[:], wo_d.ap()[128 * kk2:128 * (kk2 + 1), :])
                    t = cpool.tile([128, D], f32r, tag=f"wo{kk2}", name=f"wo{kk2}")
                    nc.vector.tensor_copy(t[:], st[:])
                    wo_sb.append(t)
                for nm, d in (("bq", bq_d), ("bk", bk_d), ("bv", bv_d)):
                    st = wpool.tile([1, FG], f32, name="bs", bufs=3)
                    nc.sync.dma_start(st[:], d.ap()[:, :])
                    t = cpool.tile([1, FG], f32r, tag=nm, name=nm)
                    nc.vector.tensor_copy(t[:], st[:])
                    b_sb[nm] = t

            qT = [cpool.tile([128, S], f32r, tag=f"qT{p}", name=f"qT{p}")
                  for p in range(2)]
            kT = [cpool.tile([128, S], f32r, tag=f"kT{p}", name=f"kT{p}")
                  for p in range(2)]
            vT = [cpool.tile([128, S], f32, tag=f"vT{p}", name=f"vT{p}")
                  for p in range(2)]
            vI = [cpool.tile([128, 4, 65], f32r, tag=f"vI{sc}", name=f"vI{sc}")
                  for sc in range(16)]
            for sc in range(16):
                nc.vector.tensor_copy(vI[sc][:, :, 64:65], ones41[:])
            ctxN = [cpool.tile([128, S], f32r, tag=f"ctxN{p}", name=f"ctxN{p}")
                    for p in range(2)]

            # ---------------- projections (8 psum banks, kk-outer) -----------
            with tc.tile_pool(name="xp", bufs=1) as xpool, \
                 tc.tile_pool(name="pp", bufs=1, space="PSUM") as ppool:

                def project(x_d, wname, bname, outT):
                    ps = [ppool.tile([128, 512], f32, name=f"pp{i}", bufs=1)
                          for i in range(8)]
                    for kk in range(8):
                        xs = xpool.tile([128, S], f32, name="xs", bufs=2)
                        nc.sync.dma_start(xs[:], x_d.ap()[128 * kk:128 * (kk + 1), :])
                        xt = xpool.tile([128, S], f32r, name="xt", bufs=2)
                        nc.gpsimd.tensor_copy(xt[:], xs[:])
                        for pch in range(2):
                            for qc in range(4):
                                i = pch * 4 + qc
                                nc.tensor.matmul(
                                    ps[i][:],
                                    w_sb[(wname, kk)][:, 128 * pch:128 * (pch + 1)],
                                    xt[:, 512 * qc:512 * (qc + 1)],
                                    start=(kk == 0), stop=False)
                    for pch in range(2):
                        for qc in range(4):
                            i = pch * 4 + qc
                            nc.tensor.matmul(
                                ps[i][:],
                                b_sb[bname][:, 128 * pch:128 * (pch + 1)],
                                ones[:, :],
                                start=False, stop=True)
                            nc.scalar.copy(outT[pch][:, 512 * qc:512 * (qc + 1)],
                                           ps[i][:])

                project(xk_d, "wk", "bk", kT)
                project(xv_d, "wv", "bv", vT)
                for pch in range(2):
                    for sc in range(16):
                        # ping-pong transpose scratch over the pp6/pp7 slots
                        tp = ppool.tile([128, 128], f32, name=f"pp{6 + sc % 2}",
                                        bufs=1)
                        nc.tensor.transpose(tp[:], vT[pch][:, 128 * sc:128 * (sc + 1)],
                                            ident[:])
                        for hh in range(2):
                            nc.vector.tensor_copy(vI[sc][:, 2 * pch + hh, 0:64],
                                                  tp[:, 64 * hh:64 * (hh + 1)])
                project(xq_d, "wq", "bq", qT)

            # ---------------- attention + output projection ------------------
            with tc.tile_pool(name="scp", bufs=3, space="PSUM") as scp, \
                 tc.tile_pool(name="cxp", bufs=2, space="PSUM") as cxp, \
                 tc.tile_pool(name="bcp", bufs=1, space="PSUM") as bcp, \
                 tc.tile_pool(name="opp", bufs=2, space="PSUM") as opp, \
                 tc.tile_pool(name="exp", bufs=3) as expool, \
                 tc.tile_pool(name="rcp", bufs=2) as rcpool, \
                 tc.tile_pool(name="bsp", bufs=2) as bspool, \
                 tc.tile_pool(name="obp", bufs=2) as obpool:
                for qj in range(4):
                    for h in range(4):
                        pch, off = h // 2, 64 * (h % 2)
                        ctx = cxp.tile([65, 512], f32, name="ctx")
                        for ki in range(16):
                            sp = scp.tile([128, 512], f32, name="sp")
                            nc.tensor.matmul(
                                sp[:],
                                kT[pch][off:off + 64, 128 * ki:128 * (ki + 1)],
                                qT[pch][off:off + 64, 512 * qj:512 * (qj + 1)],
                                start=True, stop=True)
                            ex = expool.tile([128, 512], f32r, name="ex")
                            nc.scalar.activation(ex[:], sp[:], EXP, bias=zbias[:],
                                                 scale=0.125)
                            nc.tensor.matmul(
                                ctx[:],
                                vI[ki][:, h, :],
                                ex[:],
                                start=(ki == 0), stop=(ki == 15))
                        rc = rcpool.tile([1, 512], f32r, name="rc")
                        with nc.allow_low_precision(reason="f32r for PE broadcast"):
                            nc.vector.reciprocal(rc[:], ctx[64:65, :])
                        bc = bcp.tile([64, 512], f32, name="bc")
                        nc.tensor.matmul(bc[:], ones[:, 0:64], rc[:],
                                         start=True, stop=True)
                        bcs = bspool.tile([64, 512], f32, name="bcs")
                        nc.vector.tensor_copy(bcs[:], bc[:])
                        nc.vector.tensor_tensor(
                            ctxN[pch][off:off + 64, 512 * qj:512 * (qj + 1)],
                            ctx[0:64, :], bcs[:], MULT)
                    for m in range(8):
                        op = opp.tile([128, 512], f32, name="op")
                        for kk2 in range(2):
                            nc.tensor.matmul(
                                op[:],
                                wo_sb[kk2][:, 128 * m:128 * (m + 1)],
                                ctxN[kk2][:, 512 * qj:512 * (qj + 1)],
                                start=(kk2 == 0), stop=(kk2 == 1))
                        ob = obpool.tile([128, 512], f32, name="ob")
                        nc.vector.tensor_copy(ob[:], op[:])
                        nc.sync.dma_start(
                            out_d.ap()[128 * m:128 * (m + 1), 512 * qj:512 * (qj + 1)],
                            ob[:])

    nc.compile()
    return nc


def _make_in_maps(q, k, v, wq, bq, wk, bk, wv, bv, wo):
    in_maps = []
    for c in range(8):
        b, g = divmod(c, 4)
        fs = slice(FG * g, FG * (g + 1))
        in_maps.append({
            "xq": np.ascontiguousarray(q[b].T),
            "xk": np.ascontiguousarray(k[b].T),
            "xv": np.ascontiguousarray(v[b].T),
            "wq": np.ascontiguousarray(wq[fs, :].T),
            "wk": np.ascontiguousarray(wk[fs, :].T),
            "wv": np.ascontiguousarray(wv[fs, :].T),
            "wo": np.ascontiguousarray(wo[:, fs].T),
            "bq": np.ascontiguousarray(bq[fs].reshape(1, FG)),
            "bk": np.ascontiguousarray(bk[fs].reshape(1, FG)),
            "bv": np.ascontiguousarray(bv[fs].reshape(1, FG)),
        })
    return in_maps


def kernel(q, k, v, wq, bq, wk, bk, wv, bv, wo, bo):
    from concourse.bass_utils import run_bass_kernel_spmd

    global _compiled
    if _compiled is None:
        _compiled = _build_program()
    nc = _compiled

    args = [np.asarray(a, dtype=np.float32)
            for a in (q, k, v, wq, bq, wk, bk, wv, bv, wo)]
    bo = np.asarray(bo, dtype=np.float32)
    in_maps = _make_in_maps(*args)
    res = run_bass_kernel_spmd(nc, in_maps, core_ids=list(range(8)))
    outs = [np.asarray(res.results[c]["out"]) for c in range(8)]
    full = []
    for b in range(B):
        acc = outs[4 * b] + outs[4 * b + 1] + outs[4 * b + 2] + outs[4 * b + 3]
        full.append(acc.T + bo[None, :])
    return np.stack(full).astype(np.float32)


# revision 17
# speedup vs baseline: 1.5531x; 1.5531x over previous
"""MultiHeadAttention (B=2, S=2048, D=1024, H=16, depth=64) on 8 trn2 cores.

Sharding: core c -> batch b=c//4, head-group g=c%4 (heads 4g..4g+3).
Per-core device program (SPMD, identical program, different inputs):
  - inputs pre-transposed AND cast to bf16 on host: xq/xk/xv = x_b.T
    [1024, 2048]; weights bf16; biases fp32 column vectors [256, 1].
  - Q/K/V projections feature-major (PSUM fp32 accumulation over 8 k-chunks);
    per-partition bias folded into the ACT PSUM->SBUF copy (Identity+bias).
  - V transposed on-device (PE transpose) into seq-major interleaved tiles
    vI[sc] [128, 4, 65] with an all-ones column 64 per head, so attn@V also
    produces the softmax denominator (row 64 of ctx psum).
  - attention processes head PAIRS with a one-ki software-pipeline skew:
    scores(ki) for both heads issue before attn@V(ki-1), so the PE never
    stalls waiting for the ACT exp.  exp on ACT (scale 1/8, no max
    subtraction: scores ~ N(0,1)).
  - normalization: reciprocal of ctx row 64 (DVE, f32r) -> rank-1 PE matmul
    broadcast -> multiply (DVE) into feature-major bf16 ctxN [256, 2048].
  - output projection -> outT partial [1024, 2048] fp32; host sums the 4
    head-group partials per batch, transposes back, adds bo.
bf16 operands enable FWL (fast weight load) on the PE and halve input DMA.
"""

import numpy as np
import ml_dtypes

B, S, D = 2, 2048, 1024
FG = 256  # features per core (4 heads x 64)

_compiled = None


def _build_program(repeat=1, do_proj=True, do_attn=True, do_exp=True):
    import concourse.bass as bass  # noqa: F401
    import concourse.tile as tile
    from concourse import bacc, mybir, masks

    f32 = mybir.dt.float32
    f32r = mybir.dt.float32r
    bf16 = mybir.dt.bfloat16
    EXP = mybir.ActivationFunctionType.Exp
    MULT = mybir.AluOpType.mult

    nc = bacc.Bacc("TRN2", target_bir_lowering=False, debug=False)

    xq_d = nc.dram_tensor("xq", [D, S], bf16, kind="ExternalInput")
    xk_d = nc.dram_tensor("xk", [D, S], bf16, kind="ExternalInput")
    xv_d = nc.dram_tensor("xv", [D, S], bf16, kind="ExternalInput")
    wq_d = nc.dram_tensor("wq", [D, FG], bf16, kind="ExternalInput")
    wk_d = nc.dram_tensor("wk", [D, FG], bf16, kind="ExternalInput")
    wv_d = nc.dram_tensor("wv", [D, FG], bf16, kind="ExternalInput")
    wo_d = nc.dram_tensor("wo", [FG, D], bf16, kind="ExternalInput")
    bq_d = nc.dram_tensor("bq", [FG, 1], f32, kind="ExternalInput")
    bk_d = nc.dram_tensor("bk", [FG, 1], f32, kind="ExternalInput")
    bv_d = nc.dram_tensor("bv", [FG, 1], f32, kind="ExternalInput")
    out_d = nc.dram_tensor("out", [D, S], f32, kind="ExternalOutput")

    with tile.TileContext(nc) as tc:
      for _rep in range(repeat):
        with tc.tile_pool(name="const", bufs=1) as cpool:
            onesf = cpool.tile([1, 512], f32, tag="onesf", name="onesf")
            nc.gpsimd.memset(onesf[:], 1.0)
            ones_r = cpool.tile([1, 64], f32r, tag="ones_r", name="ones_r")
            nc.vector.tensor_copy(ones_r[:], onesf[:, 0:64])
            o41f = cpool.tile([128, 4, 1], f32, tag="o41f", name="o41f")
            nc.gpsimd.memset(o41f[:], 1.0)
            ones41 = cpool.tile([128, 4, 1], bf16, tag="ones41", name="ones41")
            nc.vector.tensor_copy(ones41[:], o41f[:])
            zbias = cpool.tile([128, 1], f32, tag="zbias", name="zbias")
            nc.gpsimd.memset(zbias[:], 0.0)
            ident = cpool.tile([128, 128], bf16, tag="ident", name="ident")
            masks.make_identity(nc, ident[:])

            # weights / biases: DMA straight into resident bf16 tiles
            w_sb = {}
            wo_sb = []
            b_sb = {}
            if do_proj:
                for nm, d in (("wq", wq_d), ("wk", wk_d), ("wv", wv_d)):
                    for kk in range(8):
                        t = cpool.tile([128, FG], bf16, tag=f"{nm}{kk}",
                                       name=f"{nm}{kk}")
                        nc.sync.dma_start(t[:], d.ap()[128 * kk:128 * (kk + 1), :])
                        w_sb[(nm, kk)] = t
                for nm, d in (("bq", bq_d), ("bk", bk_d), ("bv", bv_d)):
                    for pch in range(2):
                        t = cpool.tile([128, 1], f32, tag=f"{nm}{pch}",
                                       name=f"{nm}{pch}")
                        nc.sync.dma_start(t[:], d.ap()[128 * pch:128 * (pch + 1), :])
                        b_sb[(nm, pch)] = t
            for kk2 in range(2):
                t = cpool.tile([128, D], bf16, tag=f"wo{kk2}", name=f"wo{kk2}")
                nc.sync.dma_start(t[:], wo_d.ap()[128 * kk2:128 * (kk2 + 1), :])
                wo_sb.append(t)

            qT = [cpool.tile([128, S], bf16, tag=f"qT{p}", name=f"qT{p}")
                  for p in range(2)]
            kT = [cpool.tile([128, S], bf16, tag=f"kT{p}", name=f"kT{p}")
                  for p in range(2)]
            vT = [cpool.tile([128, S], bf16, tag=f"vT{p}", name=f"vT{p}")
                  for p in range(2)]
            vI = [cpool.tile([128, 4, 65], bf16, tag=f"vI{sc}", name=f"vI{sc}")
                  for sc in range(16)]
            for sc in range(16):
                nc.vector.tensor_copy(vI[sc][:, :, 64:65], ones41[:])
            ctxN = [cpool.tile([128, S], bf16, tag=f"ctxN{p}", name=f"ctxN{p}")
                    for p in range(2)]

            if not do_proj:
                # probe mode: zero-fill qT/kT/vI
                zst = cpool.tile([128, S], f32, tag="zst", name="zst")
                nc.gpsimd.memset(zst[:], 0.0)
                for p in range(2):
                    nc.gpsimd.tensor_copy(qT[p][:], zst[:])
                    nc.gpsimd.tensor_copy(kT[p][:], zst[:])
                for sc in range(16):
                    for hh in range(4):
                        nc.vector.tensor_copy(vI[sc][:, hh, 0:64],
                                              zst[:, 0:64])

            # ---------------- projections (8 psum banks, kk-outer) -----------
            if do_proj:
              with tc.tile_pool(name="xp", bufs=1) as xpool, \
                 tc.tile_pool(name="pp", bufs=1, space="PSUM") as ppool:

                def project(x_d, wname, bname, outT):
                    ps = [ppool.tile([128, 512], f32, name=f"pp{i}", bufs=1)
                          for i in range(8)]
                    for kk in range(8):
                        xs = xpool.tile([128, S], bf16, name="xs", bufs=3)
                        nc.sync.dma_start(xs[:], x_d.ap()[128 * kk:128 * (kk + 1), :])
                        for pch in range(2):
                            for qc in range(4):
                                i = pch * 4 + qc
                                nc.tensor.matmul(
                                    ps[i][:],
                                    w_sb[(wname, kk)][:, 128 * pch:128 * (pch + 1)],
                                    xs[:, 512 * qc:512 * (qc + 1)],
                                    start=(kk == 0), stop=(kk == 7))
                    for pch in range(2):
                        for qc in range(4):
                            i = pch * 4 + qc
                            nc.scalar.add(outT[pch][:, 512 * qc:512 * (qc + 1)],
                                          ps[i][:], b_sb[(bname, pch)][:, :])

                project(xk_d, "wk", "bk", kT)
                project(xv_d, "wv", "bv", vT)
                for pch in range(2):
                    for sc in range(16):
                        # ping-pong transpose scratch over the pp6/pp7 slots
                        tp = ppool.tile([128, 128], bf16, name=f"pp{6 + sc % 2}",
                                        bufs=1)
                        nc.tensor.transpose(tp[:], vT[pch][:, 128 * sc:128 * (sc + 1)],
                                            ident[:])
                        for hh in range(2):
                            nc.vector.tensor_copy(vI[sc][:, 2 * pch + hh, 0:64],
                                                  tp[:, 64 * hh:64 * (hh + 1)])
                project(xq_d, "wq", "bq", qT)

            # ---------------- attention + output projection ------------------
            if do_attn:
              with tc.tile_pool(name="scp", bufs=1, space="PSUM") as scp, \
                 tc.tile_pool(name="cxp", bufs=1, space="PSUM") as cxp, \
                 tc.tile_pool(name="opp", bufs=2, space="PSUM") as opp, \
                 tc.tile_pool(name="exp", bufs=6) as expool, \
                 tc.tile_pool(name="rcp", bufs=2) as rcpool, \
                 tc.tile_pool(name="bsp", bufs=2) as bspool, \
                 tc.tile_pool(name="obp", bufs=2) as obpool:
                for qj in range(4):
                    for hp in range(2):
                        pch = hp
                        ctxs = [cxp.tile([65, 512], f32, name=f"ctx{hh}", bufs=1)
                                for hh in range(2)]
                        pend = None
                        for ki in range(16):
                            cur = []
                            for hh in range(2):
                                off = 64 * hh
                                sp = scp.tile([128, 512], f32, name="sp", bufs=3)
                                nc.tensor.matmul(
                                    sp[:],
                                    kT[pch][off:off + 64, 128 * ki:128 * (ki + 1)],
                                    qT[pch][off:off + 64, 512 * qj:512 * (qj + 1)],
                                    start=True, stop=True)
                                if do_exp:
                                    ex = expool.tile([128, 512], bf16, name="ex",
                                                     bufs=6)
                                    nc.scalar.activation(ex[:], sp[:], EXP,
                                                         bias=zbias[:], scale=0.125)
                                    cur.append(ex[:])
                                else:
                                    cur.append(
                                        qT[pch][:, 512 * qj:512 * (qj + 1)])
                            if pend is not None:
                                for hh in range(2):
                                    nc.tensor.matmul(
                                        ctxs[hh][:],
                                        vI[ki - 1][:, 2 * pch + hh, :],
                                        pend[hh],
                                        start=(ki == 1), stop=False)
                            pend = cur
                        for hh in range(2):
                            nc.tensor.matmul(
                                ctxs[hh][:], vI[15][:, 2 * pch + hh, :], pend[hh],
                                start=False, stop=True)
                        for hh in range(2):
                            off = 64 * hh
                            rc = rcpool.tile([1, 512], f32r, name="rc", bufs=2)
                            with nc.allow_low_precision(
                                    reason="f32r for PE broadcast"):
                                nc.vector.reciprocal(rc[:], ctxs[hh][64:65, :])
                            bc = scp.tile([64, 512], f32, name="bc", bufs=1)
                            nc.tensor.matmul(bc[:], ones_r[:, :], rc[:],
                                             start=True, stop=True)
                            bcs = bspool.tile([64, 512], f32, name="bcs", bufs=2)
                            nc.vector.tensor_copy(bcs[:], bc[:])
                            nc.vector.tensor_tensor(
                                ctxN[pch][off:off + 64, 512 * qj:512 * (qj + 1)],
                                ctxs[hh][0:64, :], bcs[:], MULT)
                    for m in range(8):
                        op = opp.tile([128, 512], f32, name="op", bufs=2)
                        for kk2 in range(2):
                            nc.tensor.matmul(
                                op[:],
                                wo_sb[kk2][:, 128 * m:128 * (m + 1)],
                                ctxN[kk2][:, 512 * qj:512 * (qj + 1)],
                                start=(kk2 == 0), stop=(kk2 == 1))
                        ob = obpool.tile([128, 512], f32, name="ob", bufs=2)
                        nc.vector.tensor_copy(ob[:], op[:])
                        nc.sync.dma_start(
                            out_d.ap()[128 * m:128 * (m + 1), 512 * qj:512 * (qj + 1)],
                            ob[:])

    nc.compile()
    return nc


def _make_in_maps(q, k, v, wq, bq, wk, bk, wv, bv, wo):
    bf = ml_dtypes.bfloat16
    in_maps = []
    for c in range(8):
        b, g = divmod(c, 4)
        fs = slice(FG * g, FG * (g + 1))
        in_maps.append({
            "xq": np.ascontiguousarray(q[b].T.astype(bf)),
            "xk": np.ascontiguousarray(k[b].T.astype(bf)),
            "xv": np.ascontiguousarray(v[b].T.astype(bf)),
            "wq": np.ascontiguousarray(wq[fs, :].T.astype(bf)),
            "wk": np.ascontiguousarray(wk[fs, :].T.astype(bf)),
            "wv": np.ascontiguousarray(wv[fs, :].T.astype(bf)),
            "wo": np.ascontiguousarray(wo[:, fs].T.astype(bf)),
            "bq": np.ascontiguousarray(bq[fs].reshape(FG, 1).astype(np.float32)),
            "bk": np.ascontiguousarray(bk[fs].reshape(FG, 1).astype(np.float32)),
            "bv": np.ascontiguousarray(bv[fs].reshape(FG, 1).astype(np.float32)),
        })
    return in_maps


def kernel(q, k, v, wq, bq, wk, bk, wv, bv, wo, bo):
    from concourse.bass_utils import run_bass_kernel_spmd

    global _compiled
    if _compiled is None:
        _compiled = _build_program()
    nc = _compiled

    args = [np.asarray(a, dtype=np.float32)
            for a in (q, k, v, wq, bq, wk, bk, wv, bv, wo)]
    bo = np.asarray(bo, dtype=np.float32)
    in_maps = _make_in_maps(*args)
    res = run_bass_kernel_spmd(nc, in_maps, core_ids=list(range(8)))
    outs = [np.asarray(res.results[c]["out"]) for c in range(8)]
    full = []
    for b in range(B):
        acc = outs[4 * b] + outs[4 * b + 1] + outs[4 * b + 2] + outs[4 * b + 3]
        full.append(acc.T + bo[None, :])
    return np.stack(full).astype(np.float32)
